# revision 10
# baseline (speedup 1.0000x reference)
"""Trainium2 Bass kernel for nn_ManifoldSKI.

Model: z <- z + gate * (tanh(sum_k a_bk (W_k z + U_k h + V_k f)) - z)
iterated to a fixed point, then a large vocab decode z* @ dec_w.T.

Key structural facts exploited:
  - The gate cannot move the fixed point (the update vanishes at z*), and
    the ungated map z <- tanh(sum_k a_k W_k z + c) is a strong contraction
    (max_b ||sum_k a_bk W_k||_2 ~= 0.25): it converges ~10x per iteration
    from z=0. So the stabilizer network is never evaluated at all, and
    ~6 iterations reach the fixed point to ~2e-5 even in fp32r.
  - The reference's own 40 damped iterations stop within 2.5e-5 of that
    same fixed point.
  - The constant term c folds to host-side O(K^2 D^2) work via the rank-2
    structure of h_ctx and the K-entry f_emb codebook.

Layout / sharding:
  - Every core runs the (cheap) fixed point for the FULL batch B=512 in
    transposed layout zT [D, B], with the K weight matmuls in fp32r
    (single-pass, full rate at free-dim 512).
  - The big memory-bound decode is sharded over the vocab: each core owns
    a 6284-column slice of dec_w.T (loaded once at startup, ~3.2 MB) and
    writes logits[:, slice]; host concatenates. No collectives needed.
"""

import os
import numpy as np

B = 512
D = 128
K = 11
V = 50257
NCORES = 8
VS = 6284          # per-core vocab slice (8*6284 = 50272, zero-padded)
BETA = 5.0
ITERS = 5          # plain-map fp32r iterations (first is tanh(c) if z0=0)
DEC_CHUNK = 2048
PSUM_N = 512
DEC_F32R = os.environ.get("DEC_F32R", "1") == "1"

_built = {}


def _build(z0_zero):
    import concourse.bass as bass
    import concourse.mybir as mybir
    import concourse.tile as tile
    from concourse import bacc

    F32 = mybir.dt.float32
    F32R = mybir.dt.float32r
    AF = mybir.ActivationFunctionType
    ALU = mybir.AluOpType
    DEC_DT = F32R if DEC_F32R else F32

    nc = bacc.Bacc("TRN2", target_bir_lowering=False, debug=False,
                   num_devices=NCORES)

    d_ct = nc.dram_tensor("cT", [D, B], F32, kind="ExternalInput")
    d_zt0 = nc.dram_tensor("zT0", [D, B], F32, kind="ExternalInput")
    d_wtr = nc.dram_tensor("WTr", [D, K * D], F32R, kind="ExternalInput")
    d_wtb = nc.dram_tensor("WTb", [D, K * D], mybir.dt.bfloat16,
                           kind="ExternalInput")
    d_abc = nc.dram_tensor("A_bc", [D, K * B], F32, kind="ExternalInput")
    d_decw = nc.dram_tensor("dec_wT", [D, VS], DEC_DT, kind="ExternalInput")
    d_out = nc.dram_tensor("logits", [B, VS], F32, kind="ExternalOutput")

    with tile.TileContext(nc) as tc:
        with tc.tile_pool(name="const", bufs=1) as cp, \
             tc.tile_pool(name="xts", bufs=4) as xp, \
             tc.tile_pool(name="zs", bufs=2) as zp, \
             tc.tile_pool(name="dop", bufs=6) as dop:

            # critical-path loads first (cT, WTr), then A_bc, then the
            # decoder slice (only needed after the fixed point).
            t_c = cp.tile([D, B], F32, name="t_c", tag="cT")
            nc.sync.dma_start(t_c[:], d_ct[:])
            BF16 = mybir.dt.bfloat16
            t_wtb = cp.tile([D, K * D], BF16, name="t_wtb", tag="wtb")
            nc.sync.dma_start(t_wtb[:], d_wtb[:])
            t_wtr = cp.tile([D, K * D], F32R, name="t_wtr", tag="wtr")
            nc.sync.dma_start(t_wtr[:], d_wtr[:])
            if not z0_zero:
                t_zt0 = cp.tile([D, B], F32, name="t_zt0", tag="zt0")
                nc.sync.dma_start(t_zt0[:], d_zt0[:])
            t_abc = cp.tile([D, K * B], F32, name="t_abc", tag="abc")
            for k in range(K):
                eng = nc.gpsimd if k % 2 == 0 else nc.sync
                eng.dma_start(t_abc[:, k * B:(k + 1) * B],
                              d_abc[:, k * B:(k + 1) * B])
            t_dec = cp.tile([D, VS], DEC_DT, name="t_dec", tag="dec")
            half = (VS // 2) & ~1
            nc.gpsimd.dma_start(t_dec[:, :half], d_decw[:, :half])
            nc.sync.dma_start(t_dec[:, half:], d_decw[:, half:])

            with tc.tile_pool(name="ps", bufs=2, space="PSUM") as pp:
                if z0_zero:
                    # z1 = tanh(c) exactly (z_init is all zeros)
                    z_cur = zp.tile([D, B], F32, name="z1", tag="z")
                    nc.scalar.activation(z_cur[:], t_c[:], AF.Tanh)
                    n_mm_iters = ITERS - 1
                else:
                    z_cur = t_zt0
                    n_mm_iters = ITERS

                for it in range(n_mm_iters):
                    # the first matmul iteration needs no precision (its
                    # error is contracted 10x by each later iteration):
                    # run it in bf16, which also warms the PE clock gate.
                    bf = it == 0
                    it_dt = BF16 if bf else F32R
                    t_w = t_wtb if bf else t_wtr
                    pre = pp.tile([D, B], F32, name=f"pre{it}", tag="pre")
                    for k in range(K):
                        xt = xp.tile([D, B], it_dt, name=f"xt{it}_{k}",
                                     tag="xtb" if bf else "xt")
                        nc.vector.tensor_tensor(
                            xt[:], t_abc[:, k * B:(k + 1) * B], z_cur[:],
                            ALU.mult)
                        nc.tensor.matmul(pre[:], t_w[:, k * D:(k + 1) * D],
                                         xt[:], start=(k == 0),
                                         stop=(k == K - 1))
                    prec = zp.tile([D, B], F32, name=f"prec{it}",
                                   tag="prec")
                    nc.vector.tensor_add(prec[:], pre[:], t_c[:])
                    last = it == n_mm_iters - 1
                    znext = zp.tile([D, B], DEC_DT if last else F32,
                                    name=f"z{it + 2}", tag="zd" if last
                                    else "z")
                    nc.scalar.activation(znext[:], prec[:], AF.Tanh)
                    z_cur = znext

            zdec = z_cur

            with tc.tile_pool(name="dps", bufs=6, space="PSUM") as dps:
                nchunk = (VS + DEC_CHUNK - 1) // DEC_CHUNK
                dma_i = 0
                for bi in range(B // D):           # 4 batch chunks of 128
                    zb = zdec[:, bi * D:(bi + 1) * D]
                    for ci in range(nchunk):
                        v0 = ci * DEC_CHUNK
                        w = min(DEC_CHUNK, VS - v0)
                        ot = dop.tile([D, DEC_CHUNK], F32,
                                      name=f"do{bi}_{ci}", tag="do")
                        for j in range(0, w, PSUM_N):
                            jw = min(PSUM_N, w - j)
                            mm_jw = jw + (jw & 1) if DEC_F32R else jw
                            ps = dps.tile([D, PSUM_N], F32,
                                          name=f"dps{bi}_{ci}_{j}",
                                          tag="dps")
                            nc.tensor.matmul(
                                ps[:, :mm_jw], zb,
                                t_dec[:, v0 + j:v0 + j + mm_jw],
                                start=True, stop=True)
                            if (j // PSUM_N) % 2 == 0:
                                nc.vector.tensor_copy(ot[:, j:j + jw],
                                                      ps[:, :jw])
                            else:
                                nc.scalar.copy(ot[:, j:j + jw],
                                               ps[:, :jw])
                        eng = nc.sync if dma_i % 2 == 0 else nc.gpsimd
                        dma_i += 1
                        eng.dma_start(
                            d_out[bi * D:(bi + 1) * D, v0:v0 + w],
                            ot[:, :w])

    nc.finalize()
    return nc


def _prep(inputs):
    """Host-side input prep + sharding. Returns (in_maps, dec_b, z0_zero)."""
    f = lambda name: np.ascontiguousarray(np.asarray(inputs[name]),
                                          dtype=np.float32)
    depth, complexity = f("depth"), f("complexity")
    z_init, op_emb, addr = f("z_init"), f("op_emb"), f("addr")
    W, U, Vm = f("W"), f("U"), f("Vm")
    dep_w, dep_b = f("dep_w"), f("dep_b")
    comp_w, comp_b = f("comp_w"), f("comp_b")
    dec_w, dec_b = f("dec_w"), f("dec_b")
    op_ids = np.asarray(inputs["op_ids"]).astype(np.int64)

    h_ctx = (depth @ dep_w.T + dep_b + complexity @ comp_w.T + comp_b)
    al = BETA * (h_ctx @ addr.T)                              # [B, K]
    al = al - al.max(axis=-1, keepdims=True)
    e = np.exp(al)
    a = e / e.sum(axis=-1, keepdims=True)                     # [B, K]

    # c = sum_k a_k*(U_k h + V_k f): rank-2 h_ctx + K-entry f_emb codebook
    f64 = np.float64
    u1 = np.einsum("kij,j->ki", U.astype(f64), dep_w[:, 0].astype(f64))
    u2 = np.einsum("kij,j->ki", U.astype(f64), comp_w[:, 0].astype(f64))
    ub = np.einsum("kij,j->ki", U.astype(f64),
                   (dep_b + comp_b).astype(f64))
    vf = np.einsum("kij,mj->kmi", Vm.astype(f64), op_emb.astype(f64))
    a64 = a.astype(f64)
    c_term = (depth.astype(f64) * (a64 @ u1)
              + complexity.astype(f64) * (a64 @ u2)
              + a64 @ ub
              + np.einsum("bk,kbi->bi", a64, vf[:, op_ids, :]))
    c_term = c_term.astype(np.float32)

    c = np.ascontiguousarray
    dec_wT_pad = np.zeros((D, NCORES * VS), np.float32)
    dec_wT_pad[:, :V] = dec_w.T
    shared = {
        "cT": c(c_term.T),
        "zT0": c(z_init.T),
        "WTr": c(W.transpose(2, 0, 1).reshape(D, K * D)),
        "WTb": np.ascontiguousarray(
            W.transpose(2, 0, 1).reshape(D, K * D).astype(
                __import__("ml_dtypes").bfloat16)),
        "A_bc": c(np.broadcast_to(
            a.T.reshape(1, K * B), (D, K * B))),
    }
    in_maps = []
    for s in range(NCORES):
        im = dict(shared)
        im["dec_wT"] = c(dec_wT_pad[:, s * VS:(s + 1) * VS])
        in_maps.append(im)
    z0_zero = not np.any(z_init)
    return in_maps, dec_b, z0_zero


def _run(inputs, trace=False, **kw):
    from concourse.bass_utils import run_bass_kernel_spmd
    in_maps, dec_b, z0_zero = _prep(inputs)
    if z0_zero not in _built:
        _built[z0_zero] = _build(z0_zero)
    res = run_bass_kernel_spmd(_built[z0_zero], in_maps,
                               core_ids=list(range(NCORES)),
                               trace=trace, **kw)
    logits = np.concatenate([r["logits"] for r in res.results],
                            axis=1)[:, :V]
    if np.any(dec_b):
        logits = logits + dec_b[None, :]
    return np.ascontiguousarray(logits), res


def kernel(**inputs) -> np.ndarray:
    logits, _ = _run(inputs, trace=False)
    return logits


# revision 11
# speedup vs baseline: 1.0578x; 1.0578x over previous
"""Trainium2 Bass kernel for nn_ManifoldSKI.

Model: z <- z + gate * (tanh(sum_k a_bk (W_k z + U_k h + V_k f)) - z)
iterated to a fixed point, then a large vocab decode z* @ dec_w.T.

Key structural facts exploited:
  - The gate cannot move the fixed point (the update vanishes at z*), and
    the ungated map z <- tanh(sum_k a_k W_k z + c) is a strong contraction
    (max_b ||sum_k a_bk W_k||_2 ~= 0.25): it converges ~10x per iteration
    from z=0. So the stabilizer network is never evaluated at all, and
    ~6 iterations reach the fixed point to ~2e-5 even in fp32r.
  - The reference's own 40 damped iterations stop within 2.5e-5 of that
    same fixed point.
  - The constant term c folds to host-side O(K^2 D^2) work via the rank-2
    structure of h_ctx and the K-entry f_emb codebook.

Layout / sharding:
  - Every core runs the (cheap) fixed point for the FULL batch B=512 in
    transposed layout zT [D, B], with the K weight matmuls in fp32r
    (single-pass, full rate at free-dim 512).
  - The big memory-bound decode is sharded over the vocab: each core owns
    a 6284-column slice of dec_w.T (loaded once at startup, ~3.2 MB) and
    writes logits[:, slice]; host concatenates. No collectives needed.
"""

import os
import numpy as np

B = 512
D = 128
K = 11
V = 50257
NCORES = 8
VS = 6284          # per-core vocab slice (8*6284 = 50272, zero-padded)
BETA = 5.0
ITERS = 5          # plain-map fp32r iterations (first is tanh(c) if z0=0)
DEC_CHUNK = 2048
PSUM_N = 512
DEC_F32R = os.environ.get("DEC_F32R", "1") == "1"

_built = {}


def _build(z0_zero):
    import concourse.bass as bass
    import concourse.mybir as mybir
    import concourse.tile as tile
    from concourse import bacc

    F32 = mybir.dt.float32
    F32R = mybir.dt.float32r
    AF = mybir.ActivationFunctionType
    ALU = mybir.AluOpType
    DEC_DT = F32R if DEC_F32R else F32

    nc = bacc.Bacc("TRN2", target_bir_lowering=False, debug=False,
                   num_devices=NCORES)

    d_ct = nc.dram_tensor("cT", [D, B], F32, kind="ExternalInput")
    d_zt0 = nc.dram_tensor("zT0", [D, B], F32, kind="ExternalInput")
    d_wtr = nc.dram_tensor("WTr", [D, K * D], F32R, kind="ExternalInput")
    d_wtb = nc.dram_tensor("WTb", [D, K * D], mybir.dt.bfloat16,
                           kind="ExternalInput")
    d_abc = nc.dram_tensor("A_bc", [D, K * B], F32, kind="ExternalInput")
    d_decw = nc.dram_tensor("dec_wT", [D, VS], DEC_DT, kind="ExternalInput")
    d_out = nc.dram_tensor("logits", [B, VS], F32, kind="ExternalOutput")

    with tile.TileContext(nc) as tc:
        with tc.tile_pool(name="const", bufs=1) as cp, \
             tc.tile_pool(name="xts", bufs=6) as xp, \
             tc.tile_pool(name="zs", bufs=2) as zp, \
             tc.tile_pool(name="dop", bufs=6) as dop:

            # critical-path loads first (cT, WTr), then A_bc, then the
            # decoder slice (only needed after the fixed point).
            # Load order matters: the SDMA engines round-robin across
            # queues at packet granularity, so anything queued anywhere
            # competes for wire. Fixed-point inputs go first on both
            # queues (A_bc chunk k in k-order so iteration 1 can chase
            # arrivals); the decoder slice (needed ~50us later) goes last.
            BF16 = mybir.dt.bfloat16
            t_c = cp.tile([D, B], F32, name="t_c", tag="cT")
            nc.sync.dma_start(t_c[:], d_ct[:])
            t_wtb = cp.tile([D, K * D], BF16, name="t_wtb", tag="wtb")
            nc.gpsimd.dma_start(t_wtb[:], d_wtb[:])
            if not z0_zero:
                t_zt0 = cp.tile([D, B], F32, name="t_zt0", tag="zt0")
                nc.sync.dma_start(t_zt0[:], d_zt0[:])
            t_abc = cp.tile([D, K * B], F32, name="t_abc", tag="abc")
            for k in range(K):
                eng = nc.sync if k % 2 == 0 else nc.gpsimd
                eng.dma_start(t_abc[:, k * B:(k + 1) * B],
                              d_abc[:, k * B:(k + 1) * B])
            t_wtr = cp.tile([D, K * D], F32R, name="t_wtr", tag="wtr")
            nc.gpsimd.dma_start(t_wtr[:], d_wtr[:])
            t_dec = cp.tile([D, VS], DEC_DT, name="t_dec", tag="dec")
            half = (VS // 2) & ~1
            nc.sync.dma_start(t_dec[:, :half], d_decw[:, :half])
            nc.gpsimd.dma_start(t_dec[:, half:], d_decw[:, half:])

            with tc.tile_pool(name="ps", bufs=2, space="PSUM") as pp:
                if z0_zero:
                    # z1 = tanh(c) exactly (z_init is all zeros)
                    z_cur = zp.tile([D, B], F32, name="z1", tag="z")
                    nc.scalar.activation(z_cur[:], t_c[:], AF.Tanh)
                    n_mm_iters = ITERS - 1
                else:
                    z_cur = t_zt0
                    n_mm_iters = ITERS

                for it in range(n_mm_iters):
                    # the first matmul iteration needs no precision (its
                    # error is contracted 10x by each later iteration):
                    # run it in bf16, which also warms the PE clock gate.
                    bf = it == 0
                    it_dt = BF16 if bf else F32R
                    t_w = t_wtb if bf else t_wtr
                    pre = pp.tile([D, B], F32, name=f"pre{it}", tag="pre")
                    for k in range(K):
                        xt = xp.tile([D, B], it_dt, name=f"xt{it}_{k}",
                                     tag="xtb" if bf else "xt")
                        nc.vector.tensor_tensor(
                            xt[:], t_abc[:, k * B:(k + 1) * B], z_cur[:],
                            ALU.mult)
                        nc.tensor.matmul(pre[:], t_w[:, k * D:(k + 1) * D],
                                         xt[:], start=(k == 0),
                                         stop=(k == K - 1))
                    prec = zp.tile([D, B], F32, name=f"prec{it}",
                                   tag="prec")
                    nc.vector.tensor_add(prec[:], pre[:], t_c[:])
                    last = it == n_mm_iters - 1
                    znext = zp.tile([D, B], DEC_DT if last else F32,
                                    name=f"z{it + 2}", tag="zd" if last
                                    else "z")
                    nc.scalar.activation(znext[:], prec[:], AF.Tanh)
                    z_cur = znext

            zdec = z_cur

            with tc.tile_pool(name="dps", bufs=6, space="PSUM") as dps:
                nchunk = (VS + DEC_CHUNK - 1) // DEC_CHUNK
                dma_i = 0
                for bi in range(B // D):           # 4 batch chunks of 128
                    zb = zdec[:, bi * D:(bi + 1) * D]
                    for ci in range(nchunk):
                        v0 = ci * DEC_CHUNK
                        w = min(DEC_CHUNK, VS - v0)
                        ot = dop.tile([D, DEC_CHUNK], F32,
                                      name=f"do{bi}_{ci}", tag="do")
                        for j in range(0, w, PSUM_N):
                            jw = min(PSUM_N, w - j)
                            mm_jw = jw + (jw & 1) if DEC_F32R else jw
                            ps = dps.tile([D, PSUM_N], F32,
                                          name=f"dps{bi}_{ci}_{j}",
                                          tag="dps")
                            nc.tensor.matmul(
                                ps[:, :mm_jw], zb,
                                t_dec[:, v0 + j:v0 + j + mm_jw],
                                start=True, stop=True)
                            if (j // PSUM_N) % 2 == 0:
                                nc.vector.tensor_copy(ot[:, j:j + jw],
                                                      ps[:, :jw])
                            else:
                                nc.scalar.copy(ot[:, j:j + jw],
                                               ps[:, :jw])
                        eng = nc.sync if dma_i % 2 == 0 else nc.gpsimd
                        dma_i += 1
                        eng.dma_start(
                            d_out[bi * D:(bi + 1) * D, v0:v0 + w],
                            ot[:, :w])

    nc.finalize()
    return nc


def _prep(inputs):
    """Host-side input prep + sharding. Returns (in_maps, dec_b, z0_zero)."""
    f = lambda name: np.ascontiguousarray(np.asarray(inputs[name]),
                                          dtype=np.float32)
    depth, complexity = f("depth"), f("complexity")
    z_init, op_emb, addr = f("z_init"), f("op_emb"), f("addr")
    W, U, Vm = f("W"), f("U"), f("Vm")
    dep_w, dep_b = f("dep_w"), f("dep_b")
    comp_w, comp_b = f("comp_w"), f("comp_b")
    dec_w, dec_b = f("dec_w"), f("dec_b")
    op_ids = np.asarray(inputs["op_ids"]).astype(np.int64)

    h_ctx = (depth @ dep_w.T + dep_b + complexity @ comp_w.T + comp_b)
    al = BETA * (h_ctx @ addr.T)                              # [B, K]
    al = al - al.max(axis=-1, keepdims=True)
    e = np.exp(al)
    a = e / e.sum(axis=-1, keepdims=True)                     # [B, K]

    # c = sum_k a_k*(U_k h + V_k f): rank-2 h_ctx + K-entry f_emb codebook
    f64 = np.float64
    u1 = np.einsum("kij,j->ki", U.astype(f64), dep_w[:, 0].astype(f64))
    u2 = np.einsum("kij,j->ki", U.astype(f64), comp_w[:, 0].astype(f64))
    ub = np.einsum("kij,j->ki", U.astype(f64),
                   (dep_b + comp_b).astype(f64))
    vf = np.einsum("kij,mj->kmi", Vm.astype(f64), op_emb.astype(f64))
    a64 = a.astype(f64)
    c_term = (depth.astype(f64) * (a64 @ u1)
              + complexity.astype(f64) * (a64 @ u2)
              + a64 @ ub
              + np.einsum("bk,kbi->bi", a64, vf[:, op_ids, :]))
    c_term = c_term.astype(np.float32)

    c = np.ascontiguousarray
    dec_wT_pad = np.zeros((D, NCORES * VS), np.float32)
    dec_wT_pad[:, :V] = dec_w.T
    shared = {
        "cT": c(c_term.T),
        "zT0": c(z_init.T),
        "WTr": c(W.transpose(2, 0, 1).reshape(D, K * D)),
        "WTb": np.ascontiguousarray(
            W.transpose(2, 0, 1).reshape(D, K * D).astype(
                __import__("ml_dtypes").bfloat16)),
        "A_bc": c(np.broadcast_to(
            a.T.reshape(1, K * B), (D, K * B))),
    }
    in_maps = []
    for s in range(NCORES):
        im = dict(shared)
        im["dec_wT"] = c(dec_wT_pad[:, s * VS:(s + 1) * VS])
        in_maps.append(im)
    z0_zero = not np.any(z_init)
    return in_maps, dec_b, z0_zero


def _run(inputs, trace=False, **kw):
    from concourse.bass_utils import run_bass_kernel_spmd
    in_maps, dec_b, z0_zero = _prep(inputs)
    if z0_zero not in _built:
        _built[z0_zero] = _build(z0_zero)
    res = run_bass_kernel_spmd(_built[z0_zero], in_maps,
                               core_ids=list(range(NCORES)),
                               trace=trace, **kw)
    logits = np.concatenate([r["logits"] for r in res.results],
                            axis=1)[:, :V]
    if np.any(dec_b):
        logits = logits + dec_b[None, :]
    return np.ascontiguousarray(logits), res


def kernel(**inputs) -> np.ndarray:
    logits, _ = _run(inputs, trace=False)
    return logits


# revision 12
# speedup vs baseline: 1.1097x; 1.0491x over previous
"""Trainium2 Bass kernel for nn_ManifoldSKI.

Model: z <- z + gate * (tanh(sum_k a_bk (W_k z + U_k h + V_k f)) - z)
iterated to a fixed point, then a large vocab decode z* @ dec_w.T.

Key structural facts exploited:
  - The gate cannot move the fixed point (the update vanishes at z*), and
    the ungated map z <- tanh(sum_k a_k W_k z + c) is a strong contraction
    (max_b ||sum_k a_bk W_k||_2 ~= 0.25): it converges ~10x per iteration
    from z=0. So the stabilizer network is never evaluated at all, and
    ~6 iterations reach the fixed point to ~2e-5 even in fp32r.
  - The reference's own 40 damped iterations stop within 2.5e-5 of that
    same fixed point.
  - The constant term c folds to host-side O(K^2 D^2) work via the rank-2
    structure of h_ctx and the K-entry f_emb codebook.

Layout / sharding:
  - Every core runs the (cheap) fixed point for the FULL batch B=512 in
    transposed layout zT [D, B], with the K weight matmuls in fp32r
    (single-pass, full rate at free-dim 512).
  - The big memory-bound decode is sharded over the vocab: each core owns
    a 6284-column slice of dec_w.T (loaded once at startup, ~3.2 MB) and
    writes logits[:, slice]; host concatenates. No collectives needed.
"""

import os
import numpy as np

B = 512
D = 128
K = 11
V = 50257
NCORES = 8
VS = 6284          # per-core vocab slice (8*6284 = 50272, zero-padded)
BETA = 5.0
ITERS = 5          # plain-map fp32r iterations (first is tanh(c) if z0=0)
DEC_CHUNK = 2048
PSUM_N = 512
DEC_F32R = os.environ.get("DEC_F32R", "1") == "1"

_built = {}


def _build(z0_zero):
    import concourse.bass as bass
    import concourse.mybir as mybir
    import concourse.tile as tile
    from concourse import bacc

    F32 = mybir.dt.float32
    F32R = mybir.dt.float32r
    AF = mybir.ActivationFunctionType
    ALU = mybir.AluOpType
    DEC_DT = F32R if DEC_F32R else F32

    nc = bacc.Bacc("TRN2", target_bir_lowering=False, debug=False,
                   num_devices=NCORES)

    d_ct = nc.dram_tensor("cT", [D, B], F32, kind="ExternalInput")
    d_zt0 = nc.dram_tensor("zT0", [D, B], F32, kind="ExternalInput")
    d_wtr = nc.dram_tensor("WTr", [D, K * D], F32R, kind="ExternalInput")
    d_wtb = nc.dram_tensor("WTb", [D, K * D], mybir.dt.bfloat16,
                           kind="ExternalInput")
    d_abc = nc.dram_tensor("A_bc", [D, K * B], F32, kind="ExternalInput")
    d_abcb = nc.dram_tensor("A_bcb", [D, K * B], mybir.dt.bfloat16,
                            kind="ExternalInput")
    d_decw = nc.dram_tensor("dec_wT", [D, VS], DEC_DT, kind="ExternalInput")
    d_out = nc.dram_tensor("logits", [B, VS], F32, kind="ExternalOutput")

    with tile.TileContext(nc) as tc:
        with tc.tile_pool(name="const", bufs=1) as cp, \
             tc.tile_pool(name="xts", bufs=6) as xp, \
             tc.tile_pool(name="zs", bufs=2) as zp, \
             tc.tile_pool(name="dop", bufs=6) as dop:

            # critical-path loads first (cT, WTr), then A_bc, then the
            # decoder slice (only needed after the fixed point).
            # Load order matters: the SDMA engines round-robin across
            # queues at packet granularity, so anything queued anywhere
            # competes for wire. Fixed-point inputs go first on both
            # queues (A_bc chunk k in k-order so iteration 1 can chase
            # arrivals); the decoder slice (needed ~50us later) goes last.
            BF16 = mybir.dt.bfloat16
            t_c = cp.tile([D, B], F32, name="t_c", tag="cT")
            nc.sync.dma_start(t_c[:], d_ct[:])
            t_wtb = cp.tile([D, K * D], BF16, name="t_wtb", tag="wtb")
            nc.gpsimd.dma_start(t_wtb[:], d_wtb[:])
            if not z0_zero:
                t_zt0 = cp.tile([D, B], F32, name="t_zt0", tag="zt0")
                nc.sync.dma_start(t_zt0[:], d_zt0[:])
            # bf16 op-address weights feed the early bf16 iterations and
            # arrive fast; the fp32 copy is only needed by the final
            # fp32r iteration ~30us in, so it loads in the background.
            t_abcb = cp.tile([D, K * B], BF16, name="t_abcb", tag="abcb")
            hb = (K * B) // 2
            nc.sync.dma_start(t_abcb[:, :hb], d_abcb[:, :hb])
            nc.gpsimd.dma_start(t_abcb[:, hb:], d_abcb[:, hb:])
            t_abc = cp.tile([D, K * B], F32, name="t_abc", tag="abc")
            nc.sync.dma_start(t_abc[:, :hb], d_abc[:, :hb])
            nc.gpsimd.dma_start(t_abc[:, hb:], d_abc[:, hb:])
            t_wtr = cp.tile([D, K * D], F32R, name="t_wtr", tag="wtr")
            nc.gpsimd.dma_start(t_wtr[:], d_wtr[:])
            t_dec = cp.tile([D, VS], DEC_DT, name="t_dec", tag="dec")
            half = (VS // 2) & ~1
            nc.sync.dma_start(t_dec[:, :half], d_decw[:, :half])
            nc.gpsimd.dma_start(t_dec[:, half:], d_decw[:, half:])

            with tc.tile_pool(name="ps", bufs=2, space="PSUM") as pp:
                if z0_zero:
                    # z1 = tanh(c) exactly (z_init is all zeros)
                    z_cur = zp.tile([D, B], F32, name="z1", tag="z")
                    nc.scalar.activation(z_cur[:], t_c[:], AF.Tanh)
                    n_mm_iters = ITERS - 1
                else:
                    z_cur = t_zt0
                    n_mm_iters = ITERS

                for it in range(n_mm_iters):
                    # all but the last iteration run in bf16: each later
                    # iteration contracts incoming error ~10x, so only the
                    # final fp32r pass sets the converged precision.
                    bf = it < n_mm_iters - 1
                    it_dt = BF16 if bf else F32R
                    t_w = t_wtb if bf else t_wtr
                    t_a = t_abcb if bf else t_abc
                    pre = pp.tile([D, B], F32, name=f"pre{it}", tag="pre")
                    for k in range(K):
                        xt = xp.tile([D, B], it_dt, name=f"xt{it}_{k}",
                                     tag="xtb" if bf else "xt")
                        nc.vector.tensor_tensor(
                            xt[:], t_a[:, k * B:(k + 1) * B], z_cur[:],
                            ALU.mult)
                        nc.tensor.matmul(pre[:], t_w[:, k * D:(k + 1) * D],
                                         xt[:], start=(k == 0),
                                         stop=(k == K - 1))
                    prec = zp.tile([D, B], F32, name=f"prec{it}",
                                   tag="prec")
                    nc.vector.tensor_add(prec[:], pre[:], t_c[:])
                    last = it == n_mm_iters - 1
                    znext = zp.tile([D, B], DEC_DT if last else F32,
                                    name=f"z{it + 2}", tag="zd" if last
                                    else "z")
                    nc.scalar.activation(znext[:], prec[:], AF.Tanh)
                    z_cur = znext

            zdec = z_cur

            with tc.tile_pool(name="dps", bufs=6, space="PSUM") as dps:
                nchunk = (VS + DEC_CHUNK - 1) // DEC_CHUNK
                dma_i = 0
                for bi in range(B // D):           # 4 batch chunks of 128
                    zb = zdec[:, bi * D:(bi + 1) * D]
                    for ci in range(nchunk):
                        v0 = ci * DEC_CHUNK
                        w = min(DEC_CHUNK, VS - v0)
                        ot = dop.tile([D, DEC_CHUNK], F32,
                                      name=f"do{bi}_{ci}", tag="do")
                        for j in range(0, w, PSUM_N):
                            jw = min(PSUM_N, w - j)
                            mm_jw = jw + (jw & 1) if DEC_F32R else jw
                            ps = dps.tile([D, PSUM_N], F32,
                                          name=f"dps{bi}_{ci}_{j}",
                                          tag="dps")
                            nc.tensor.matmul(
                                ps[:, :mm_jw], zb,
                                t_dec[:, v0 + j:v0 + j + mm_jw],
                                start=True, stop=True)
                            if (j // PSUM_N) % 2 == 0:
                                nc.vector.tensor_copy(ot[:, j:j + jw],
                                                      ps[:, :jw])
                            else:
                                nc.scalar.copy(ot[:, j:j + jw],
                                               ps[:, :jw])
                        eng = nc.sync if dma_i % 2 == 0 else nc.gpsimd
                        dma_i += 1
                        eng.dma_start(
                            d_out[bi * D:(bi + 1) * D, v0:v0 + w],
                            ot[:, :w])

    nc.finalize()
    return nc


def _prep(inputs):
    """Host-side input prep + sharding. Returns (in_maps, dec_b, z0_zero)."""
    f = lambda name: np.ascontiguousarray(np.asarray(inputs[name]),
                                          dtype=np.float32)
    depth, complexity = f("depth"), f("complexity")
    z_init, op_emb, addr = f("z_init"), f("op_emb"), f("addr")
    W, U, Vm = f("W"), f("U"), f("Vm")
    dep_w, dep_b = f("dep_w"), f("dep_b")
    comp_w, comp_b = f("comp_w"), f("comp_b")
    dec_w, dec_b = f("dec_w"), f("dec_b")
    op_ids = np.asarray(inputs["op_ids"]).astype(np.int64)

    h_ctx = (depth @ dep_w.T + dep_b + complexity @ comp_w.T + comp_b)
    al = BETA * (h_ctx @ addr.T)                              # [B, K]
    al = al - al.max(axis=-1, keepdims=True)
    e = np.exp(al)
    a = e / e.sum(axis=-1, keepdims=True)                     # [B, K]

    # c = sum_k a_k*(U_k h + V_k f): rank-2 h_ctx + K-entry f_emb codebook
    f64 = np.float64
    u1 = np.einsum("kij,j->ki", U.astype(f64), dep_w[:, 0].astype(f64))
    u2 = np.einsum("kij,j->ki", U.astype(f64), comp_w[:, 0].astype(f64))
    ub = np.einsum("kij,j->ki", U.astype(f64),
                   (dep_b + comp_b).astype(f64))
    vf = np.einsum("kij,mj->kmi", Vm.astype(f64), op_emb.astype(f64))
    a64 = a.astype(f64)
    c_term = (depth.astype(f64) * (a64 @ u1)
              + complexity.astype(f64) * (a64 @ u2)
              + a64 @ ub
              + np.einsum("bk,kbi->bi", a64, vf[:, op_ids, :]))
    c_term = c_term.astype(np.float32)

    c = np.ascontiguousarray
    dec_wT_pad = np.zeros((D, NCORES * VS), np.float32)
    dec_wT_pad[:, :V] = dec_w.T
    shared = {
        "cT": c(c_term.T),
        "zT0": c(z_init.T),
        "WTr": c(W.transpose(2, 0, 1).reshape(D, K * D)),
        "WTb": np.ascontiguousarray(
            W.transpose(2, 0, 1).reshape(D, K * D).astype(
                __import__("ml_dtypes").bfloat16)),
        "A_bc": c(np.broadcast_to(
            a.T.reshape(1, K * B), (D, K * B))),
        "A_bcb": np.ascontiguousarray(np.broadcast_to(
            a.T.reshape(1, K * B).astype(
                __import__("ml_dtypes").bfloat16), (D, K * B))),
    }
    in_maps = []
    for s in range(NCORES):
        im = dict(shared)
        im["dec_wT"] = c(dec_wT_pad[:, s * VS:(s + 1) * VS])
        in_maps.append(im)
    z0_zero = not np.any(z_init)
    return in_maps, dec_b, z0_zero


def _run(inputs, trace=False, **kw):
    from concourse.bass_utils import run_bass_kernel_spmd
    in_maps, dec_b, z0_zero = _prep(inputs)
    if z0_zero not in _built:
        _built[z0_zero] = _build(z0_zero)
    res = run_bass_kernel_spmd(_built[z0_zero], in_maps,
                               core_ids=list(range(NCORES)),
                               trace=trace, **kw)
    logits = np.concatenate([r["logits"] for r in res.results],
                            axis=1)[:, :V]
    if np.any(dec_b):
        logits = logits + dec_b[None, :]
    return np.ascontiguousarray(logits), res


def kernel(**inputs) -> np.ndarray:
    logits, _ = _run(inputs, trace=False)
    return logits


# revision 13
# speedup vs baseline: 1.1968x; 1.0785x over previous
"""Trainium2 Bass kernel for nn_ManifoldSKI.

Model: z <- z + gate * (tanh(sum_k a_bk (W_k z + U_k h + V_k f)) - z)
iterated to a fixed point, then a large vocab decode z* @ dec_w.T.

Key structural facts exploited:
  - The gate cannot move the fixed point (the update vanishes at z*), and
    the ungated map z <- tanh(sum_k a_k W_k z + c) is a strong contraction
    (max_b ||sum_k a_bk W_k||_2 ~= 0.25): it converges ~10x per iteration
    from z=0. So the stabilizer network is never evaluated at all, and
    ~6 iterations reach the fixed point to ~2e-5 even in fp32r.
  - The reference's own 40 damped iterations stop within 2.5e-5 of that
    same fixed point.
  - The constant term c folds to host-side O(K^2 D^2) work via the rank-2
    structure of h_ctx and the K-entry f_emb codebook.

Layout / sharding:
  - Every core runs the (cheap) fixed point for the FULL batch B=512 in
    transposed layout zT [D, B], with the K weight matmuls in fp32r
    (single-pass, full rate at free-dim 512).
  - The big memory-bound decode is sharded over the vocab: each core owns
    a 6284-column slice of dec_w.T (loaded once at startup, ~3.2 MB) and
    writes logits[:, slice]; host concatenates. No collectives needed.
"""

import os
import numpy as np

B = 512
D = 128
K = 11
V = 50257
NCORES = 8
VS = 6284          # per-core vocab slice (8*6284 = 50272, zero-padded)
BETA = 5.0
ITERS = 5          # plain-map fp32r iterations (first is tanh(c) if z0=0)
DEC_CHUNK = 1024
PSUM_N = 512
DEC_F32R = os.environ.get("DEC_F32R", "1") == "1"

_built = {}


def _build(z0_zero):
    import concourse.bass as bass
    import concourse.mybir as mybir
    import concourse.tile as tile
    from concourse import bacc

    F32 = mybir.dt.float32
    F32R = mybir.dt.float32r
    AF = mybir.ActivationFunctionType
    ALU = mybir.AluOpType
    DEC_DT = F32R if DEC_F32R else F32

    nc = bacc.Bacc("TRN2", target_bir_lowering=False, debug=False,
                   num_devices=NCORES)

    d_ct = nc.dram_tensor("cT", [D, B], F32, kind="ExternalInput")
    d_zt0 = nc.dram_tensor("zT0", [D, B], F32, kind="ExternalInput")
    d_wtr = nc.dram_tensor("WTr", [D, K * D], F32R, kind="ExternalInput")
    d_wtb = nc.dram_tensor("WTb", [D, K * D], mybir.dt.bfloat16,
                           kind="ExternalInput")
    d_abc = nc.dram_tensor("A_bc", [D, K * B], F32, kind="ExternalInput")
    d_abcb = nc.dram_tensor("A_bcb", [D, K * B], mybir.dt.bfloat16,
                            kind="ExternalInput")
    d_decw = nc.dram_tensor("dec_wT", [D, VS], DEC_DT, kind="ExternalInput")
    d_out = nc.dram_tensor("logits", [B, VS], F32, kind="ExternalOutput")

    with tile.TileContext(nc) as tc:
        with tc.tile_pool(name="const", bufs=1) as cp, \
             tc.tile_pool(name="xts", bufs=6) as xp, \
             tc.tile_pool(name="zs", bufs=2) as zp, \
             tc.tile_pool(name="dop", bufs=8) as dop:

            # critical-path loads first (cT, WTr), then A_bc, then the
            # decoder slice (only needed after the fixed point).
            # Load order matters: the SDMA engines round-robin across
            # queues at packet granularity, so anything queued anywhere
            # competes for wire. Fixed-point inputs go first on both
            # queues (A_bc chunk k in k-order so iteration 1 can chase
            # arrivals); the decoder slice (needed ~50us later) goes last.
            BF16 = mybir.dt.bfloat16
            t_c = cp.tile([D, B], F32, name="t_c", tag="cT")
            nc.sync.dma_start(t_c[:], d_ct[:])
            t_wtb = cp.tile([D, K * D], BF16, name="t_wtb", tag="wtb")
            nc.gpsimd.dma_start(t_wtb[:], d_wtb[:])
            if not z0_zero:
                t_zt0 = cp.tile([D, B], F32, name="t_zt0", tag="zt0")
                nc.sync.dma_start(t_zt0[:], d_zt0[:])
            # bf16 op-address weights feed the early bf16 iterations and
            # arrive fast; the fp32 copy is only needed by the final
            # fp32r iteration ~30us in, so it loads in the background.
            t_abcb = cp.tile([D, K * B], BF16, name="t_abcb", tag="abcb")
            hb = (K * B) // 2
            nc.sync.dma_start(t_abcb[:, :hb], d_abcb[:, :hb])
            nc.gpsimd.dma_start(t_abcb[:, hb:], d_abcb[:, hb:])
            t_abc = cp.tile([D, K * B], F32, name="t_abc", tag="abc")
            nc.sync.dma_start(t_abc[:, :hb], d_abc[:, :hb])
            nc.gpsimd.dma_start(t_abc[:, hb:], d_abc[:, hb:])
            t_wtr = cp.tile([D, K * D], F32R, name="t_wtr", tag="wtr")
            nc.gpsimd.dma_start(t_wtr[:], d_wtr[:])
            t_dec = cp.tile([D, VS], DEC_DT, name="t_dec", tag="dec")
            half = (VS // 2) & ~1
            nc.sync.dma_start(t_dec[:, :half], d_decw[:, :half])
            nc.gpsimd.dma_start(t_dec[:, half:], d_decw[:, half:])

            with tc.tile_pool(name="ps", bufs=2, space="PSUM") as pp:
                if z0_zero:
                    # z1 = tanh(c) exactly (z_init is all zeros); stored
                    # bf16 since its only consumer is the bf16 iteration.
                    z_cur = zp.tile([D, B], BF16, name="z1", tag="zb")
                    nc.scalar.activation(z_cur[:], t_c[:], AF.Tanh)
                    n_mm_iters = ITERS - 1
                else:
                    z_cur = t_zt0
                    n_mm_iters = ITERS

                for it in range(n_mm_iters):
                    # all but the last iteration run in bf16: each later
                    # iteration contracts incoming error ~10x, so only the
                    # final fp32r pass sets the converged precision.
                    bf = it < n_mm_iters - 1
                    it_dt = BF16 if bf else F32R
                    t_w = t_wtb if bf else t_wtr
                    t_a = t_abcb if bf else t_abc
                    pre = pp.tile([D, B], F32, name=f"pre{it}", tag="pre")
                    for k in range(K):
                        xt = xp.tile([D, B], it_dt, name=f"xt{it}_{k}",
                                     tag="xtb" if bf else "xt")
                        nc.vector.tensor_tensor(
                            xt[:], t_a[:, k * B:(k + 1) * B], z_cur[:],
                            ALU.mult)
                        nc.tensor.matmul(pre[:], t_w[:, k * D:(k + 1) * D],
                                         xt[:], start=(k == 0),
                                         stop=(k == K - 1))
                    last = it == n_mm_iters - 1
                    # z dtype: bf16 while feeding bf16 iterations; fp32
                    # into the final fp32r iteration; decode dtype at end.
                    if last:
                        z_dt, z_tag = DEC_DT, "zd"
                    elif it == n_mm_iters - 2:
                        z_dt, z_tag = F32, "z"
                    else:
                        z_dt, z_tag = BF16, "zb"
                    prec = zp.tile([D, B], F32, name=f"prec{it}",
                                   tag="prec")
                    znext = zp.tile([D, B], z_dt, name=f"z{it + 2}",
                                    tag=z_tag)
                    hb2 = B // 2
                    for h in range(2):
                        sl = slice(h * hb2, (h + 1) * hb2)
                        nc.vector.tensor_add(prec[:, sl], pre[:, sl],
                                             t_c[:, sl])
                        nc.scalar.activation(znext[:, sl], prec[:, sl],
                                             AF.Tanh)
                    z_cur = znext

            zdec = z_cur

            with tc.tile_pool(name="dps", bufs=6, space="PSUM") as dps:
                nchunk = (VS + DEC_CHUNK - 1) // DEC_CHUNK
                dma_i = 0
                for bi in range(B // D):           # 4 batch chunks of 128
                    zb = zdec[:, bi * D:(bi + 1) * D]
                    for ci in range(nchunk):
                        v0 = ci * DEC_CHUNK
                        w = min(DEC_CHUNK, VS - v0)
                        ot = dop.tile([D, DEC_CHUNK], F32,
                                      name=f"do{bi}_{ci}", tag="do")
                        for j in range(0, w, PSUM_N):
                            jw = min(PSUM_N, w - j)
                            mm_jw = jw + (jw & 1) if DEC_F32R else jw
                            ps = dps.tile([D, PSUM_N], F32,
                                          name=f"dps{bi}_{ci}_{j}",
                                          tag="dps")
                            nc.tensor.matmul(
                                ps[:, :mm_jw], zb,
                                t_dec[:, v0 + j:v0 + j + mm_jw],
                                start=True, stop=True)
                            if (j // PSUM_N) % 2 == 0:
                                nc.vector.tensor_copy(ot[:, j:j + jw],
                                                      ps[:, :jw])
                            else:
                                nc.scalar.copy(ot[:, j:j + jw],
                                               ps[:, :jw])
                        eng = nc.sync if dma_i % 2 == 0 else nc.gpsimd
                        dma_i += 1
                        eng.dma_start(
                            d_out[bi * D:(bi + 1) * D, v0:v0 + w],
                            ot[:, :w])

    nc.finalize()
    return nc


def _prep(inputs):
    """Host-side input prep + sharding. Returns (in_maps, dec_b, z0_zero)."""
    f = lambda name: np.ascontiguousarray(np.asarray(inputs[name]),
                                          dtype=np.float32)
    depth, complexity = f("depth"), f("complexity")
    z_init, op_emb, addr = f("z_init"), f("op_emb"), f("addr")
    W, U, Vm = f("W"), f("U"), f("Vm")
    dep_w, dep_b = f("dep_w"), f("dep_b")
    comp_w, comp_b = f("comp_w"), f("comp_b")
    dec_w, dec_b = f("dec_w"), f("dec_b")
    op_ids = np.asarray(inputs["op_ids"]).astype(np.int64)

    h_ctx = (depth @ dep_w.T + dep_b + complexity @ comp_w.T + comp_b)
    al = BETA * (h_ctx @ addr.T)                              # [B, K]
    al = al - al.max(axis=-1, keepdims=True)
    e = np.exp(al)
    a = e / e.sum(axis=-1, keepdims=True)                     # [B, K]

    # c = sum_k a_k*(U_k h + V_k f): rank-2 h_ctx + K-entry f_emb codebook
    f64 = np.float64
    u1 = np.einsum("kij,j->ki", U.astype(f64), dep_w[:, 0].astype(f64))
    u2 = np.einsum("kij,j->ki", U.astype(f64), comp_w[:, 0].astype(f64))
    ub = np.einsum("kij,j->ki", U.astype(f64),
                   (dep_b + comp_b).astype(f64))
    vf = np.einsum("kij,mj->kmi", Vm.astype(f64), op_emb.astype(f64))
    a64 = a.astype(f64)
    c_term = (depth.astype(f64) * (a64 @ u1)
              + complexity.astype(f64) * (a64 @ u2)
              + a64 @ ub
              + np.einsum("bk,kbi->bi", a64, vf[:, op_ids, :]))
    c_term = c_term.astype(np.float32)

    c = np.ascontiguousarray
    dec_wT_pad = np.zeros((D, NCORES * VS), np.float32)
    dec_wT_pad[:, :V] = dec_w.T
    shared = {
        "cT": c(c_term.T),
        "zT0": c(z_init.T),
        "WTr": c(W.transpose(2, 0, 1).reshape(D, K * D)),
        "WTb": np.ascontiguousarray(
            W.transpose(2, 0, 1).reshape(D, K * D).astype(
                __import__("ml_dtypes").bfloat16)),
        "A_bc": c(np.broadcast_to(
            a.T.reshape(1, K * B), (D, K * B))),
        "A_bcb": np.ascontiguousarray(np.broadcast_to(
            a.T.reshape(1, K * B).astype(
                __import__("ml_dtypes").bfloat16), (D, K * B))),
    }
    in_maps = []
    for s in range(NCORES):
        im = dict(shared)
        im["dec_wT"] = c(dec_wT_pad[:, s * VS:(s + 1) * VS])
        in_maps.append(im)
    z0_zero = not np.any(z_init)
    return in_maps, dec_b, z0_zero


def _run(inputs, trace=False, **kw):
    from concourse.bass_utils import run_bass_kernel_spmd
    in_maps, dec_b, z0_zero = _prep(inputs)
    if z0_zero not in _built:
        _built[z0_zero] = _build(z0_zero)
    res = run_bass_kernel_spmd(_built[z0_zero], in_maps,
                               core_ids=list(range(NCORES)),
                               trace=trace, **kw)
    logits = np.concatenate([r["logits"] for r in res.results],
                            axis=1)[:, :V]
    if np.any(dec_b):
        logits = logits + dec_b[None, :]
    return np.ascontiguousarray(logits), res


def kernel(**inputs) -> np.ndarray:
    logits, _ = _run(inputs, trace=False)
    return logits
